# revision 1
# baseline (speedup 1.0000x reference)
"""Trainium2 Bass kernel for the contrastive loss problem.

Strategy (8 NeuronCores, SPMD):
  - Core c receives `features` rotated by -1024*c rows, so each core's
    "own" rows are local rows 0..1023 and the single compiled program is
    identical across cores.
  - On device: normalize rows -> z (f32), cast to bf16, transpose via
    TensorE to zT [D, rows].  Each core computes its [1024, 8192] slice
    of E = exp((z @ z.T) / tau) with bf16 matmuls; the ScalarE activation
    computes exp with a fused row-sum (accum_out).  The numerically
    sensitive same-group sums live in the 128x128 diagonal blocks, which
    are recomputed in fp32 and reduced with a host-supplied block-diag
    mask.
  - Host combines per-core row sums:  pos = S - e^(1/tau), neg = T - S,
    loss = mean(log(neg) - log(pos)).
"""

import sys

import numpy as np

sys.path.insert(0, "/opt/trn_rl_repo")

N, D = 8192, 128
NCORES = 8
RPC = N // NCORES  # rows per core (1024)
CHUNKS = N // 128  # 64 row-chunks of 128
ACH = RPC // 128  # own row-chunks per core (8)
TAU = 0.5
EPS = 1e-8

_PROGRAM = None
_COMPILE_PATCHED = False


def _patch_compile():
    """This container's walrus build rejects two instructions that the Tile
    framework emits in its kernel tail: a Drain carrying more than one sem
    wait ("Too many sync wait commands") and the EVENT_SEMAPHORE_RANGE_CLEAR
    ISA instruction ("ISA wrong length").  Rewrite the BIR before walrus sees
    it: split multi-wait Drains into chains of single-wait Drains, and drop
    the range-clear (sems are left dirty, so one NEFF load supports a single
    execution -- kernel() is called once per process, which is our usage)."""
    global _COMPILE_PATCHED
    if _COMPILE_PATCHED:
        return
    import orjson

    import concourse.bass2jax as bass2jax
    import concourse.bass_utils as bass_utils

    orig = bass_utils.compile_bir_kernel

    def patched(bir_json, tmpdir, neff_name="file.neff"):
        bir = orjson.loads(bir_json)
        for fn in bir.get("functions", []):
            for bb in fn.get("blocks", []):
                new_insts = []
                for ins in bb.get("instructions", []):
                    if (
                        ins.get("opcode") == "ISA"
                        and ins.get("isa_opcode") == 176
                    ):
                        continue  # EVENT_SEMAPHORE_RANGE_CLEAR
                    sync = ins.get("sync_info")
                    if sync and len(sync.get("on_wait") or []) > 1:
                        waits = sync["on_wait"]
                        for k, w in enumerate(waits[:-1]):
                            pre = {
                                "engine": ins["engine"],
                                "name": f"{ins['name']}_w{k}",
                                "opcode": "Drain",
                                "ins": [],
                                "outs": [],
                                "sync_info": {"on_update": [], "on_wait": [w]},
                            }
                            if "debug" in ins:
                                pre["debug"] = ins["debug"]
                            new_insts.append(pre)
                        sync["on_wait"] = [waits[-1]]
                    new_insts.append(ins)
                bb["instructions"] = new_insts
        return orig(orjson.dumps(bir), tmpdir, neff_name=neff_name)

    bass_utils.compile_bir_kernel = patched
    bass2jax.compile_bir_kernel = patched
    _COMPILE_PATCHED = True


def _build_program():
    import concourse.bass as bass
    import concourse.mybir as mybir
    import concourse.tile as tile
    from concourse.masks import make_identity

    f32 = mybir.dt.float32
    bf16 = mybir.dt.bfloat16
    AF = mybir.ActivationFunctionType
    AX = mybir.AxisListType
    OP = mybir.AluOpType

    nc = bass.Bass("TRN2", target_bir_lowering=False, debug=False)

    feat = nc.dram_tensor("feat", [N, D], f32, kind="ExternalInput")
    maskd = nc.dram_tensor("mask", [128, 128], f32, kind="ExternalInput")
    t_out = nc.dram_tensor("t_out", [RPC], f32, kind="ExternalOutput")
    tdb_out = nc.dram_tensor("tdb_out", [RPC], f32, kind="ExternalOutput")
    tdf_out = nc.dram_tensor("tdf_out", [RPC], f32, kind="ExternalOutput")
    s_out = nc.dram_tensor("s_out", [RPC], f32, kind="ExternalOutput")

    # DRAM view: row = k*128 + p  ->  [p, k, d]
    feat_r = feat.ap().rearrange("(k p) d -> p k d", p=128)

    G8 = 8  # chunks per DMA/transform group

    with tile.TileContext(nc) as tc:
        with (
            tc.tile_pool(name="singles", bufs=1) as singles,
            tc.tile_pool(name="fload", bufs=3) as fload,
            tc.tile_pool(name="zstage", bufs=2) as zstage,
            tc.tile_pool(name="scratch", bufs=2) as scratch,
            tc.tile_pool(name="eslab", bufs=3) as eslab,
            tc.tile_pool(name="ptr", bufs=2, space="PSUM") as ptr,
            tc.tile_pool(name="ptr32", bufs=2, space="PSUM") as ptr32,
            tc.tile_pool(name="pmain", bufs=2, space="PSUM") as pmain,
        ):
            # ---- constants / persistent buffers ----
            idn_bf = singles.tile([128, 128], bf16)
            make_identity(nc, idn_bf[:])
            idn_32 = singles.tile([128, 128], f32)
            make_identity(nc, idn_32[:])
            mask_sb = singles.tile([128, 128], f32)
            nc.sync.dma_start(mask_sb[:], maskd.ap())

            zT_bf = singles.tile([128, N], bf16)  # [d, local rows]
            zT_32 = singles.tile([128, RPC], f32)  # own chunks, fp32
            ss = singles.tile([128, CHUNKS], f32)
            nrm = singles.tile([128, CHUNKS], f32)
            rcp = singles.tile([128, CHUNKS], f32)
            tacc = singles.tile([128, ACH * 8], f32)
            t_sb = singles.tile([128, ACH], f32)
            tdb_sb = singles.tile([128, ACH], f32)
            tdf_sb = singles.tile([128, ACH], f32)
            s_sb = singles.tile([128, ACH], f32)
            ediag = singles.tile([128, ACH, 128], f32)

            # ---- phase 1: load, normalize, transpose ----
            for g in range(CHUNKS // G8):
                g0 = g * G8
                Fg = fload.tile([128, G8, 128], f32)
                nc.sync.dma_start(Fg[:], feat_r[:, g0 : g0 + G8, :])

                sq = scratch.tile([128, G8, 128], f32, tag="sq")
                nc.vector.tensor_mul(sq[:], Fg[:], Fg[:])
                nc.vector.reduce_sum(
                    out=ss[:, g0 : g0 + G8], in_=sq[:], axis=AX.X
                )
                nc.scalar.sqrt(nrm[:, g0 : g0 + G8], ss[:, g0 : g0 + G8])
                nc.vector.tensor_scalar_max(
                    nrm[:, g0 : g0 + G8], nrm[:, g0 : g0 + G8], EPS
                )
                nc.vector.reciprocal(rcp[:, g0 : g0 + G8], nrm[:, g0 : g0 + G8])

                z8 = zstage.tile([128, G8, 128], bf16, tag="zbf")
                for i in range(G8):
                    nc.vector.tensor_scalar_mul(
                        z8[:, i, :], Fg[:, i, :], rcp[:, g0 + i : g0 + i + 1]
                    )
                trp = ptr.tile([128, G8, 128], bf16)
                for i in range(G8):
                    nc.tensor.transpose(trp[:, i, :], z8[:, i, :], idn_bf[:])
                nc.vector.tensor_copy(
                    zT_bf[:, g0 * 128 : (g0 + G8) * 128],
                    trp[:].rearrange("p a b -> p (a b)"),
                )

                if g == 0:
                    # fp32 z for the own (diagonal) chunks
                    z832 = zstage.tile([128, G8, 128], f32, tag="z32")
                    for i in range(G8):
                        nc.vector.tensor_scalar_mul(
                            z832[:, i, :], Fg[:, i, :], rcp[:, i : i + 1]
                        )
                    for r in range(2):
                        trp32 = ptr32.tile([128, 4, 128], f32, tag="p32")
                        for i in range(4):
                            nc.tensor.transpose(
                                trp32[:, i, :], z832[:, r * 4 + i, :], idn_32[:]
                            )
                        nc.vector.tensor_copy(
                            zT_32[:, r * 512 : (r + 1) * 512],
                            trp32[:].rearrange("p a b -> p (a b)"),
                        )
                    # fp32 diagonal blocks: gram, exp, masked sums
                    for r in range(2):
                        dps = ptr32.tile([128, 4, 128], f32, tag="p32")
                        for i in range(4):
                            A = r * 4 + i
                            nc.tensor.matmul(
                                dps[:, i, :],
                                zT_32[:, A * 128 : (A + 1) * 128],
                                zT_32[:, A * 128 : (A + 1) * 128],
                                start=True,
                                stop=True,
                            )
                        nc.scalar.activation(
                            out=ediag[:, r * 4 : (r + 1) * 4, :].rearrange(
                                "p a b -> p (a b)"
                            ),
                            in_=dps[:].rearrange("p a b -> p (a b)"),
                            func=AF.Exp,
                            scale=2.0,
                        )
                    nc.vector.reduce_sum(out=tdf_sb[:], in_=ediag[:], axis=AX.X)
                    for A in range(ACH):
                        mtmp = scratch.tile([128, 128], f32, tag="sq")
                        nc.vector.tensor_mul(
                            mtmp[:], ediag[:, A, :], mask_sb[:]
                        )
                        nc.vector.reduce_sum(
                            out=s_sb[:, A : A + 1], in_=mtmp[:], axis=AX.X
                        )

            # ---- phase 2: E slabs, fused exp + row sums ----
            for A in range(ACH):
                lhsT = zT_bf[:, A * 128 : (A + 1) * 128]
                for j in range(8):
                    pm = pmain.tile([128, 1024], f32)
                    for m in range(2):
                        c0 = j * 1024 + m * 512
                        nc.tensor.matmul(
                            pm[:, m * 512 : (m + 1) * 512],
                            lhsT,
                            zT_bf[:, c0 : c0 + 512],
                            start=True,
                            stop=True,
                        )
                    es = eslab.tile([128, 1024], bf16)
                    nc.scalar.activation(
                        out=es[:],
                        in_=pm[:],
                        func=AF.Exp,
                        scale=2.0,
                        accum_out=tacc[:, A * 8 + j : A * 8 + j + 1],
                    )
                    if j == 0:
                        # bf16 row-sum of the diagonal block (to be replaced
                        # by the fp32 version on host)
                        nc.vector.reduce_sum(
                            out=tdb_sb[:, A : A + 1],
                            in_=es[:, A * 128 : (A + 1) * 128],
                            axis=AX.X,
                        )

            nc.vector.reduce_sum(
                out=t_sb[:],
                in_=tacc[:].rearrange("p (a j) -> p a j", a=ACH),
                axis=AX.X,
            )

            for sb, dr in (
                (t_sb, t_out),
                (tdb_sb, tdb_out),
                (tdf_sb, tdf_out),
                (s_sb, s_out),
            ):
                nc.sync.dma_start(dr.ap().rearrange("(a p) -> p a", p=128), sb[:])

    return nc


def _get_program():
    global _PROGRAM
    if _PROGRAM is None:
        _PROGRAM = _build_program()
    return _PROGRAM


def _group_ids(num_crops):
    ids = np.repeat(np.arange(num_crops.shape[0], dtype=np.int64), num_crops)
    if ids.shape[0] >= N:
        return ids[:N]
    return np.pad(ids, (0, N - ids.shape[0]), mode="edge")


def _build_mask(num_crops):
    """[128,128] same-group mask, valid when the group pattern repeats
    every 128 rows and no group straddles a 128-row boundary."""
    ids = _group_ids(num_crops)
    pat = ids.reshape(CHUNKS, 128)
    # group-local pattern per chunk must be identical across chunks, and
    # chunks must not share groups
    local = pat - pat[:, :1]
    if not (local == local[0]).all():
        return None
    if (pat[1:, 0] <= pat[:-1, -1]).any():
        return None
    return (local[0][:, None] == local[0][None, :]).astype(np.float32)


def _numpy_fallback(feat, num_crops):
    ids = _group_ids(num_crops)
    nrm = np.maximum(np.sqrt((feat.astype(np.float64) ** 2).sum(-1)), EPS)
    z = feat / nrm[:, None].astype(np.float32)
    T = np.empty(N, np.float64)
    S = np.empty(N, np.float64)
    for r0 in range(0, N, 512):
        E = np.exp((z[r0 : r0 + 512] @ z.T) / TAU).astype(np.float64)
        same = ids[r0 : r0 + 512, None] == ids[None, :]
        T[r0 : r0 + 512] = E.sum(1)
        S[r0 : r0 + 512] = np.where(same, E, 0.0).sum(1)
    pos = S - np.exp(1.0 / TAU)
    neg = T - S
    return np.asarray(np.mean(np.log(neg) - np.log(pos)), dtype=np.float32)


def kernel(features, num_crops):
    feat = np.ascontiguousarray(np.asarray(features, dtype=np.float32))
    ncr = np.asarray(num_crops)
    mask = _build_mask(ncr)
    if mask is None:
        return _numpy_fallback(feat, ncr)

    _patch_compile()
    from concourse.bass_utils import run_bass_kernel_spmd

    nc = _get_program()
    in_maps = [
        {"feat": np.roll(feat, -RPC * c, axis=0).copy(), "mask": mask}
        for c in range(NCORES)
    ]
    res = run_bass_kernel_spmd(nc, in_maps, core_ids=list(range(NCORES)))

    T = np.empty(N, np.float64)
    S = np.empty(N, np.float64)
    for c in range(NCORES):
        r = res.results[c]
        Tc = (
            r["t_out"].astype(np.float64)
            - r["tdb_out"].astype(np.float64)
            + r["tdf_out"].astype(np.float64)
        )
        T[RPC * c : RPC * (c + 1)] = Tc
        S[RPC * c : RPC * (c + 1)] = r["s_out"].astype(np.float64)

    pos = S - np.exp(1.0 / TAU)
    neg = T - S
    loss = np.mean(np.log(neg) - np.log(pos))
    return np.asarray(loss, dtype=np.float32)



# revision 2
# speedup vs baseline: 2.7500x; 2.7500x over previous
"""Trainium2 Bass kernel for the contrastive loss problem.

Strategy (8 NeuronCores, SPMD, symmetric-half E):
  - Host normalizes features in f32 (exact norms), casts to bf16 and
    pre-transposes: zT [D=128, N] so the device does no transposes.
  - Global row-chunk g (of 64) is the anchor of exactly one core
    (core c anchors local chunks 0..7 = global 8c..8c+7).  Anchor a
    computes E blocks against column chunks a+1..a+32 only (each
    unordered chunk pair lands on exactly one anchor, except distance
    32 which both sides compute and count row-wise only).
  - Row sums come free from the ScalarE activation accumulator.
    Mirror (column) sums: DVE accumulates the fp16 E slabs into
    col_acc[128, 38*128]; the host does the final partition sum.
  - Diagonal blocks are recomputed in fp32 (pos terms need precision),
    masked-reduced on DVE for S.
  - Host combines: T = own rowsums + diag + mirror colsums,
    pos = S - e^(1/tau), neg = T - S, loss = mean(log(neg) - log(pos)).
"""

import sys

import numpy as np

sys.path.insert(0, "/opt/trn_rl_repo")

N, D = 8192, 128
NCORES = 8
RPC = N // NCORES  # rows per core (1024)
CHUNKS = N // 128  # 64 chunks
ACH = 8  # anchor chunks per core
NCOL = 32  # column chunks per anchor (distances 1..32)
MIRC = 38  # mirror chunks with col-acc (distances 1..31 -> local 1..38)
ZTC = 40  # zt column chunks a core needs (0..39)
TAU = 0.5
EPS = 1e-8

_PROGRAM = None
_COMPILE_PATCHED = False


def _patch_compile():
    """This container's walrus build rejects two instructions that the Tile
    framework emits in its kernel tail: a Drain carrying more than one sem
    wait ("Too many sync wait commands") and the EVENT_SEMAPHORE_RANGE_CLEAR
    ISA instruction ("ISA wrong length").  Rewrite the BIR before walrus sees
    it: split multi-wait Drains into chains of single-wait Drains, and drop
    the range-clear (sems are left dirty, so one NEFF load supports a single
    execution -- kernel() is called once per process, which is our usage)."""
    global _COMPILE_PATCHED
    if _COMPILE_PATCHED:
        return
    import orjson

    import concourse.bass2jax as bass2jax
    import concourse.bass_utils as bass_utils

    orig = bass_utils.compile_bir_kernel

    def patched(bir_json, tmpdir, neff_name="file.neff"):
        bir = orjson.loads(bir_json)
        for fn in bir.get("functions", []):
            for bb in fn.get("blocks", []):
                new_insts = []
                for ins in bb.get("instructions", []):
                    if (
                        ins.get("opcode") == "ISA"
                        and ins.get("isa_opcode") == 176
                    ):
                        continue  # EVENT_SEMAPHORE_RANGE_CLEAR
                    sync = ins.get("sync_info")
                    if sync and len(sync.get("on_wait") or []) > 1:
                        waits = sync["on_wait"]
                        for k, w in enumerate(waits[:-1]):
                            pre = {
                                "engine": ins["engine"],
                                "name": f"{ins['name']}_w{k}",
                                "opcode": "Drain",
                                "ins": [],
                                "outs": [],
                                "sync_info": {"on_update": [], "on_wait": [w]},
                            }
                            if "debug" in ins:
                                pre["debug"] = ins["debug"]
                            new_insts.append(pre)
                        sync["on_wait"] = [waits[-1]]
                    new_insts.append(ins)
                bb["instructions"] = new_insts
        return orig(orjson.dumps(bir), tmpdir, neff_name=neff_name)

    bass_utils.compile_bir_kernel = patched
    bass2jax.compile_bir_kernel = patched
    _COMPILE_PATCHED = True


def _build_program():
    import concourse.bass as bass
    import concourse.mybir as mybir
    import concourse.tile as tile

    f32 = mybir.dt.float32
    f16 = mybir.dt.float16
    bf16 = mybir.dt.bfloat16
    AF = mybir.ActivationFunctionType
    AX = mybir.AxisListType

    nc = bass.Bass("TRN2", target_bir_lowering=False, debug=False)

    zt_d = nc.dram_tensor("zt", [128, ZTC * 128], bf16, kind="ExternalInput")
    zt32_d = nc.dram_tensor("zt32", [128, RPC], f32, kind="ExternalInput")
    mask_d = nc.dram_tensor("mask", [128, 128], f32, kind="ExternalInput")
    pout_d = nc.dram_tensor("pout", [128, 24], f32, kind="ExternalOutput")
    cacc_d = nc.dram_tensor("cacc", [128, MIRC * 128], f16, kind="ExternalOutput")

    with tile.TileContext(nc) as tc:
        with (
            tc.tile_pool(name="singles", bufs=1) as singles,
            tc.tile_pool(name="scratch", bufs=2) as scratch,
            tc.tile_pool(name="es", bufs=3) as es,
            tc.tile_pool(name="pm", bufs=2, space="PSUM") as pm,
        ):
            zt_sb = singles.tile([128, ZTC * 128], bf16)
            zt32_sb = singles.tile([128, RPC], f32)
            mask_sb = singles.tile([128, 128], f32)
            col_acc = singles.tile([128, MIRC * 128], f16)
            ediag = singles.tile([128, RPC], f32)
            tacc = singles.tile([128, 16], f32)
            pout_sb = singles.tile([128, 24], f32)

            # ---- input DMAs: zt32+mask first (diag path), zt spread wide
            nc.scalar.dma_start(zt32_sb[:, 0:512], zt32_d.ap()[:, 0:512])
            nc.scalar.dma_start(zt32_sb[:, 512:1024], zt32_d.ap()[:, 512:1024])
            nc.scalar.dma_start(mask_sb[:], mask_d.ap())
            for i in range(10):
                c0, c1 = i * 512, (i + 1) * 512
                nc.sync.dma_start(zt_sb[:, c0:c1], zt_d.ap()[:, c0:c1])

            nc.gpsimd.memset(col_acc[:], 0.0)

            # ---- diagonal blocks in fp32 ----
            pdiag = pm.tile([128, 2048], f32, tag="pm")
            for A in range(ACH):
                zc = zt32_sb[:, A * 128 : (A + 1) * 128]
                nc.tensor.matmul(
                    pdiag[:, A * 128 : (A + 1) * 128], zc, zc,
                    start=True, stop=True,
                )
            nc.scalar.activation(
                out=ediag[:, 0:1024], in_=pdiag[:, 0:1024],
                func=AF.Exp, scale=2.0,
            )
            # tdf: per-chunk row sums of ediag
            nc.vector.reduce_sum(
                out=pout_sb[:, 8:16],
                in_=ediag.rearrange("p (a q) -> p a q", a=ACH),
                axis=AX.X,
            )
            # S: masked per-chunk sums
            for A in range(ACH):
                mtmp = scratch.tile([128, 128], f32, tag="mt")
                nc.vector.tensor_mul(
                    mtmp[:], ediag[:, A * 128 : (A + 1) * 128], mask_sb[:]
                )
                nc.vector.reduce_sum(
                    out=pout_sb[:, 16 + A : 17 + A], in_=mtmp[:], axis=AX.X
                )

            # ---- anchors: E slabs over column chunks a+1..a+32 ----
            for a in range(ACH):
                lhsT = zt_sb[:, a * 128 : (a + 1) * 128]
                for s in range(2):
                    base = (a + 1) * 128 + s * 2048
                    pt = pm.tile([128, 2048], f32, tag="pm")
                    for m in range(4):
                        nc.tensor.matmul(
                            pt[:, m * 512 : (m + 1) * 512],
                            lhsT,
                            zt_sb[:, base + m * 512 : base + (m + 1) * 512],
                            start=True, stop=True,
                        )
                    et = es.tile([128, 2048], f16)
                    nc.scalar.activation(
                        out=et[:], in_=pt[:], func=AF.Exp, scale=2.0,
                        accum_out=tacc[:, a * 2 + s : a * 2 + s + 1],
                    )
                    # mirror accumulation (exclude distance-32 chunk)
                    w = 2048 if s == 0 else 1920
                    off = a * 128 + s * 2048  # col_acc col 0 == local chunk 1
                    nc.vector.tensor_add(
                        col_acc[:, off : off + w],
                        et[:, 0:w],
                        col_acc[:, off : off + w],
                    )

            # T_main = sum of the two per-anchor slab accumulators
            nc.vector.reduce_sum(
                out=pout_sb[:, 0:8],
                in_=tacc.rearrange("p (a s) -> p a s", a=ACH),
                axis=AX.X,
            )

            # ---- outputs ----
            nc.sync.dma_start(pout_d.ap(), pout_sb[:])
            q = MIRC * 128 // 4  # 1216
            for i in range(4):
                eng = nc.sync if i % 2 == 0 else nc.scalar
                eng.dma_start(
                    cacc_d.ap()[:, i * q : (i + 1) * q],
                    col_acc[:, i * q : (i + 1) * q],
                )

    return nc


def _get_program():
    global _PROGRAM
    if _PROGRAM is None:
        _PROGRAM = _build_program()
    return _PROGRAM


def _group_ids(num_crops):
    ids = np.repeat(np.arange(num_crops.shape[0], dtype=np.int64), num_crops)
    if ids.shape[0] >= N:
        return ids[:N]
    return np.pad(ids, (0, N - ids.shape[0]), mode="edge")


def _build_mask(num_crops):
    """[128,128] same-group mask, valid when the group pattern repeats
    every 128 rows and no group straddles a 128-row boundary."""
    ids = _group_ids(num_crops)
    pat = ids.reshape(CHUNKS, 128)
    local = pat - pat[:, :1]
    if not (local == local[0]).all():
        return None
    if (pat[1:, 0] <= pat[:-1, -1]).any():
        return None
    return (local[0][:, None] == local[0][None, :]).astype(np.float32)


def _prep(feat):
    """Host prep: exact f32 normalize, bf16 cast, transpose."""
    import ml_dtypes

    nrm = np.maximum(np.sqrt((feat.astype(np.float64) ** 2).sum(-1)), EPS)
    z32 = (feat / nrm[:, None]).astype(np.float32)
    zbfT = np.ascontiguousarray(z32.astype(ml_dtypes.bfloat16).T)  # [128, N]
    z32T = np.ascontiguousarray(z32.T)  # [128, N]
    return zbfT, z32T


def _make_inmaps(feat, mask):
    zbfT, z32T = _prep(feat)
    in_maps = []
    for c in range(NCORES):
        zt = np.ascontiguousarray(
            np.roll(zbfT, -RPC * c, axis=1)[:, : ZTC * 128]
        )
        zt32 = np.ascontiguousarray(z32T[:, RPC * c : RPC * (c + 1)])
        in_maps.append({"zt": zt, "zt32": zt32, "mask": mask})
    return in_maps


def _combine(results):
    """Host combine of per-core partials -> loss (f64)."""
    T = np.zeros(N, np.float64)
    S = np.zeros(N, np.float64)
    for c in range(NCORES):
        r = results[c]
        pout = r["pout"].astype(np.float64)  # [128, 24]
        tmain, tdf, s = pout[:, 0:8], pout[:, 8:16], pout[:, 16:24]
        for a in range(ACH):
            g = 8 * c + a
            rows = slice(g * 128, (g + 1) * 128)
            T[rows] += tmain[:, a] + tdf[:, a]
            S[rows] = s[:, a]
        cs = r["cacc"].astype(np.float64).sum(axis=0).reshape(MIRC, 128)
        for j in range(1, MIRC + 1):
            g = (8 * c + j) % CHUNKS
            T[g * 128 : (g + 1) * 128] += cs[j - 1]
    pos = S - np.exp(1.0 / TAU)
    neg = T - S
    return np.asarray(np.mean(np.log(neg) - np.log(pos)), dtype=np.float32)


def _numpy_fallback(feat, num_crops):
    ids = _group_ids(num_crops)
    nrm = np.maximum(np.sqrt((feat.astype(np.float64) ** 2).sum(-1)), EPS)
    z = feat / nrm[:, None].astype(np.float32)
    T = np.empty(N, np.float64)
    S = np.empty(N, np.float64)
    for r0 in range(0, N, 512):
        E = np.exp((z[r0 : r0 + 512] @ z.T) / TAU).astype(np.float64)
        same = ids[r0 : r0 + 512, None] == ids[None, :]
        T[r0 : r0 + 512] = E.sum(1)
        S[r0 : r0 + 512] = np.where(same, E, 0.0).sum(1)
    pos = S - np.exp(1.0 / TAU)
    neg = T - S
    return np.asarray(np.mean(np.log(neg) - np.log(pos)), dtype=np.float32)


def kernel(features, num_crops):
    feat = np.ascontiguousarray(np.asarray(features, dtype=np.float32))
    ncr = np.asarray(num_crops)
    mask = _build_mask(ncr)
    if mask is None:
        return _numpy_fallback(feat, ncr)

    _patch_compile()
    from concourse.bass_utils import run_bass_kernel_spmd

    nc = _get_program()
    in_maps = _make_inmaps(feat, mask)
    res = run_bass_kernel_spmd(nc, in_maps, core_ids=list(range(NCORES)))
    return _combine(res.results)
